# revision 30
# baseline (speedup 1.0000x reference)
"""Trainium2 Bass kernel for nn_CubeSimulator (galaxy velocity-cube KDE).

Math (matches reference.py exactly, up to fp32 rounding):
    rot_x = img_x*cr - img_y*sr
    y1    = img_x*sr + img_y*cr
    rot_y = y1*ci - img_z*si
    rot_z = y1*si + img_z*ci
    r^2   = rot_x^2 + rot_y^2
    vz    = -si*v_max*rot_x / sqrt(r^2 + r_t^2)     (== -v_y*si of the ref)
    I     = exp(-(sqrt(r^2)/r_d + |rot_z|/h_z))      (I0 applied on host)
    cube[v, pix] = sum_k I * exp(-(lab_v - vz)^2 / sig^2)

Channel structure: anchor channels evaluate G_v = I * exp(-d_v^2/sig^2)
via two ScalarE passes (Square with free affine d = vz/sig - lab_v/sig,
then Exp) and one bf16 VectorE multiply by I; both factors are <= 1 so
nothing overflows and underflow-to-0 matches f32 reference semantics.
Follower channels take 1-2 multiplicative steps G_{v+-1} = G_v * t^{+-1}
with t = exp((2*dlab/sig2)*vz) (bf16 tensor_tensor, DVE 2x mode); the
channel-constant ratio exp(b_v - b_anchor) is applied on the host to the
final [V, pix] cube.  Anchors sit every 5 channels with bidirectional
chains (max 2 steps), so a chain crosses the fp32 underflow boundary by
at most ~e^32, bounding the absolute error at ~1e-24.

Distribution: the leading i-axis (192) is sharded over 8 NeuronCores
(24 planes each -> 4608 sky pixels/core, full k=192 line of sight).
Each core runs the same NEFF; in_maps differ per core.

Per-core layout: k on SBUF partitions, one merged [128, 6912] free axis:
cols 0..4607 = pixels (k 0..127), cols 4608..6911 = pixel pairs
(col c: rows 0:64 = k 128..191 of pixel c-4608, rows 64:128 = pixel
c-4608+2304).  TensorE reduces over k per 128-pixel block (lhsT = G
block as weights, rhs = ones).  PSUM cube tiles rotate over 4 banks so
the region1(start)/region2(accum) same-column pairs are spaced 4 apart
and never stall the PE pipeline; ScalarE drains every 8 channels.
"""

import contextlib

import numpy as np

import concourse.bass as bass
import concourse.bacc as bacc
import concourse.tile as tile
from concourse import mybir
from concourse.alu_op_type import AluOpType
from concourse.bass_utils import run_bass_kernel_spmd
from concourse.masks import make_identity

PC = 3.086e16
GRID = 192
N_CORES = 8
IPC = GRID // N_CORES          # 24 i-planes per core
PIX = IPC * GRID               # 4608 pixels per core
NB = PIX // 128                # 36 pixel blocks
NB2 = NB // 2                  # 18 region-2 pair blocks
K = GRID                       # 192 samples along line of sight
K1 = 128                       # region-1 k rows
PIX2 = PIX // 2                # region-2 pixel-pair columns
MCOL = PIX + PIX2              # merged free width (6912)
EPOCH = 32                     # channels per PSUM drain epoch
ANCHOR = 5                     # anchor spacing (bidirectional chains <= 2)

F32 = mybir.dt.float32
BF16 = mybir.dt.bfloat16
AF = mybir.ActivationFunctionType


def _channel_plan(V):
    """Emission-ordered plan: list of (v, kind, src_v). kind: 'a' anchor,
    'f' forward step (g_v = g_src * t), 'b' backward (g_v = g_src / t).
    Anchors at multiples of ANCHOR (plus a tail anchor); chains <= 2."""
    plan = []
    emitted = set()

    def anchor(v):
        if v not in emitted:
            plan.append((v, "a", None))
            emitted.add(v)

    for m in range(0, V, ANCHOR):
        anchor(m)
        for v in (m + 1, m + 2):
            if v < V and v not in emitted:
                plan.append((v, "f", v - 1))
                emitted.add(v)
        hi = min(m + ANCHOR, V - 1)          # next anchor or tail anchor
        if hi > m + 2:
            anchor(hi)
            for v in range(hi - 1, m + 2, -1):
                if v not in emitted:
                    plan.append((v, "b", v + 1))
                    emitted.add(v)
    assert len(plan) == V and len(emitted) == V
    return plan


def _build(nc, V, consts):
    """Emit the Tile program for one core (same program for all cores)."""
    cr, sr, ci, si = consts["cr"], consts["sr"], consts["ci"], consts["si"]
    inv_rd, inv_hz = consts["inv_rd"], consts["inv_hz"]
    cvz = consts["cvz"]                      # -si*v_max
    inv_sig = consts["inv_sig"]              # 1/sig
    st2 = consts["st2"]                      # 2*dlab/sig2
    assert V <= 64
    plan = _channel_plan(V)

    imx = nc.dram_tensor("imx", [PIX, K], F32, kind="ExternalInput").ap()
    imy = nc.dram_tensor("imy", [PIX, K], F32, kind="ExternalInput").ap()
    imz = nc.dram_tensor("imz", [PIX, K], F32, kind="ExternalInput").ap()
    # cvec: [rt2, -lab_0/sig .. -lab_{V-1}/sig] as per-partition ACT biases
    cvec = nc.dram_tensor("cvec", [1, V + 1], F32, kind="ExternalInput").ap()
    cube = nc.dram_tensor("cube", [V, PIX], F32, kind="ExternalOutput").ap()

    # pixel-major view: [partition(=pixel%128), block, k]
    vx = imx.rearrange("(g p) k -> p g k", p=128)
    vy = imy.rearrange("(g p) k -> p g k", p=128)
    vzr = imz.rearrange("(g p) k -> p g k", p=128)

    with tile.TileContext(nc) as tc:
        ctx = contextlib.ExitStack()
        with ctx:
            singles = ctx.enter_context(tc.tile_pool(name="singles", bufs=1))
            tp_psum = ctx.enter_context(
                tc.tile_pool(name="tp_psum", bufs=1, space="PSUM"))
            c_psum = ctx.enter_context(
                tc.tile_pool(name="c_psum", bufs=1, space="PSUM"))
            out_psum = ctx.enter_context(
                tc.tile_pool(name="out_psum", bufs=2, space="PSUM"))
            stage = ctx.enter_context(tc.tile_pool(name="stage", bufs=3))

            ident = singles.tile([128, 128], F32)
            make_identity(nc, ident)
            ones_a = singles.tile([128, 1], BF16)
            nc.vector.memset(ones_a, 1.0)
            # region-2 pair reduce: col0 sums rows 0:64, col1 rows 64:128
            ones_p = singles.tile([128, 2], BF16)
            nc.vector.memset(ones_p[0:64, 0:1], 1.0)
            nc.vector.memset(ones_p[64:128, 0:1], 0.0)
            nc.vector.memset(ones_p[0:64, 1:2], 0.0)
            nc.vector.memset(ones_p[64:128, 1:2], 1.0)
            cv_sb = singles.tile([128, V + 1], F32)
            nc.sync.dma_start(
                out=cv_sb,
                in_=bass.AP(tensor=cvec.tensor, offset=cvec.offset,
                            ap=[[0, 128]] + cvec.ap[1:]))

            # persistent k-major tensors (merged region1 | region2 layout)
            vzT = singles.tile([128, MCOL], F32)
            iT = singles.tile([128, MCOL], BF16)
            tT = singles.tile([128, MCOL], BF16)
            tiT = singles.tile([128, MCOL], BF16)
            cube_sb = singles.tile([128, NB, V], F32)

            # ---------------- preprocessing (scoped pools) ----------------
            pre_ctx = contextlib.ExitStack()
            with pre_ctx:
                pre2 = pre_ctx.enter_context(tc.tile_pool(name="pre2", bufs=2))
                pre1 = pre_ctx.enter_context(tc.tile_pool(name="pre1", bufs=2))
                CH = 4                       # pixel blocks per chunk
                for c0 in range(0, NB, CH):
                    ng = min(CH, NB - c0)
                    X = pre2.tile([128, ng, K], F32, tag="X")
                    Y = pre2.tile([128, ng, K], F32, tag="Y")
                    Z = pre2.tile([128, ng, K], F32, tag="Z")
                    nc.sync.dma_start(out=X, in_=vx[:, c0:c0 + ng, :])
                    nc.sync.dma_start(out=Y, in_=vy[:, c0:c0 + ng, :])
                    nc.sync.dma_start(out=Z, in_=vzr[:, c0:c0 + ng, :])

                    t0 = pre1.tile([128, ng, K], F32, tag="t0")
                    t1 = pre1.tile([128, ng, K], F32, tag="t1")
                    t2 = pre1.tile([128, ng, K], F32, tag="t2")
                    A = pre1.tile([128, ng, K], F32, tag="A")
                    RY = pre1.tile([128, ng, K], F32, tag="RY")
                    RZ = pre1.tile([128, ng, K], F32, tag="RZ")
                    WI = pre2.tile([128, ng, K], F32, tag="WI")
                    VZ = pre2.tile([128, ng, K], F32, tag="VZ")

                    # A = rot_x = cr*X - sr*Y
                    nc.vector.tensor_scalar_mul(t0, X, float(cr))
                    nc.vector.scalar_tensor_tensor(
                        A, Y, float(-sr), t0, AluOpType.mult, AluOpType.add)
                    # y1 = sr*X + cr*Y   (into t1)
                    nc.vector.tensor_scalar_mul(t2, X, float(sr))
                    nc.vector.scalar_tensor_tensor(
                        t1, Y, float(cr), t2, AluOpType.mult, AluOpType.add)
                    # RY = ci*y1 - si*Z
                    nc.vector.tensor_scalar_mul(t0, t1, float(ci))
                    nc.vector.scalar_tensor_tensor(
                        RY, Z, float(-si), t0, AluOpType.mult, AluOpType.add)
                    # RZ = si*y1 + ci*Z
                    nc.vector.tensor_scalar_mul(t2, t1, float(si))
                    nc.vector.scalar_tensor_tensor(
                        RZ, Z, float(ci), t2, AluOpType.mult, AluOpType.add)
                    # r2 = A^2 + RY^2
                    nc.vector.tensor_mul(t0, A, A)
                    nc.vector.tensor_mul(t1, RY, RY)
                    nc.vector.tensor_add(t0, t0, t1)
                    # Ln group (one ACT table set), then Exp group
                    nc.scalar.activation(
                        t1, t0, AF.Ln, bias=cv_sb[:, 0:1], scale=1.0)
                    nc.scalar.activation(t2, t0, AF.Ln, bias=0.0, scale=1.0)
                    # t1 = rsqrt(r2+rt2); t2 = r = sqrt(r2) (ln(0)->-inf->0)
                    nc.scalar.activation(t1, t1, AF.Exp, bias=0.0, scale=-0.5)
                    nc.scalar.activation(t2, t2, AF.Exp, bias=0.0, scale=0.5)
                    # VZ = cvz*A*rsqrt
                    nc.vector.scalar_tensor_tensor(
                        VZ, A, float(cvz), t1, AluOpType.mult, AluOpType.mult)
                    # t1 = |RZ|/hz ; WI = -(r/rd) - t1   (ln of intensity)
                    nc.scalar.activation(
                        t1, RZ, AF.Abs, bias=0.0, scale=float(inv_hz))
                    nc.vector.scalar_tensor_tensor(
                        WI, t2, float(-inv_rd), t1,
                        AluOpType.mult, AluOpType.subtract)

                    # transpose into the merged k-major tensors; 4 blocks
                    # batch into one PSUM bank and drain with one ScalarE
                    # op, which doubles as Exp for the intensity tensor.
                    for srcT, dT, fn in ((VZ, vzT, AF.Copy),
                                         (WI, iT, AF.Exp)):
                        p1 = tp_psum.tile([128, 4, 128], F32, tag="p1")
                        for gg in range(ng):
                            nc.tensor.transpose(
                                p1[:, gg, :], srcT[:, gg, 0:K1], ident)
                        nc.scalar.activation(
                            dT[:, c0 * 128:(c0 + ng) * 128],
                            p1[:, 0:ng, :], fn, bias=0.0, scale=1.0)
                        p2 = tp_psum.tile([64, 4, 128], F32, tag="p2")
                        for gg in range(ng):
                            nc.tensor.transpose(
                                p2[:, gg, :], srcT[:, gg, K1:K], ident)
                        # region-2 halves: block g -> rows (g<18 ? 0:64),
                        # cols PIX + (g%18)*128; split at the 18 boundary
                        lo = c0
                        while lo < c0 + ng:
                            hi = min(c0 + ng, NB2 if lo < NB2 else NB)
                            r0 = 0 if lo < NB2 else 64
                            gc = PIX + (lo % NB2) * 128
                            nc.scalar.activation(
                                dT[r0:r0 + 64, gc:gc + (hi - lo) * 128],
                                p2[0:64, lo - c0:hi - c0, :], fn,
                                bias=0.0, scale=1.0)
                            lo = hi
                    # channel-step ratio tensors for this chunk's columns
                    for dst, sc in ((tT, st2), (tiT, -st2)):
                        nc.scalar.activation(
                            dst[:, c0 * 128:(c0 + ng) * 128],
                            vzT[:, c0 * 128:(c0 + ng) * 128],
                            AF.Exp, bias=0.0, scale=float(sc))
                        lo = c0
                        while lo < c0 + ng:
                            hi = min(c0 + ng, NB2 if lo < NB2 else NB)
                            r0 = 0 if lo < NB2 else 64
                            gc = PIX + (lo % NB2) * 128
                            nc.scalar.activation(
                                dst[r0:r0 + 64, gc:gc + (hi - lo) * 128],
                                vzT[r0:r0 + 64, gc:gc + (hi - lo) * 128],
                                AF.Exp, bias=0.0, scale=float(sc))
                            lo = hi

            # ---------------- channel loop ----------------
            q_pool = ctx.enter_context(tc.tile_pool(name="q_pool", bufs=1))
            m_pool = ctx.enter_context(tc.tile_pool(name="m_pool", bufs=3))
            g_pool = ctx.enter_context(tc.tile_pool(name="g_pool", bufs=4))

            n_ep = (V + EPOCH - 1) // EPOCH
            ep_left = [min(EPOCH, V - e * EPOCH) for e in range(n_ep)]
            cpb = {}
            g_of = {}
            m_of = {}
            anchors = [v for (v, kind, _s) in plan if kind == "a"]

            def ensure_exp(av):
                if av in m_of:
                    return
                q = q_pool.tile([128, MCOL], F32, tag="q", name=f"q{av}")
                m = m_pool.tile([128, MCOL], BF16, tag="m", name=f"m{av}")
                nc.scalar.activation(
                    q, vzT, AF.Square,
                    bias=cv_sb[:, av + 1:av + 2], scale=float(inv_sig))
                nc.scalar.activation(m, q, AF.Exp, bias=0.0, scale=-1.0)
                m_of[av] = m

            for (v, kind, src) in plan:
                ep = v // EPOCH
                if ep not in cpb:
                    banks = []
                    for qi in range(4):
                        cpq = c_psum.tile([128, 10 * EPOCH], F32,
                                          tag=f"cp{qi}", name=f"cp{qi}")
                        banks.append(cpq)
                    cpb[ep] = banks
                vloc = v % EPOCH
                g = g_pool.tile([128, MCOL], BF16, tag="g")
                if kind == "a":
                    ensure_exp(v)
                    ai = anchors.index(v)
                    if ai + 1 < len(anchors):
                        ensure_exp(anchors[ai + 1])
                    nc.vector.tensor_mul(g, m_of.pop(v), iT)
                else:
                    step = tT if kind == "f" else tiT
                    nc.vector.tensor_mul(g, g_of[src], step)
                g_of[v] = g

                # Pair p2 (blocks p2 and p2+18) -> PSUM bank p2%4, col
                # pair (j=p2//4): half h at col (2j+h)*EPOCH+vloc.  Per
                # 4-group: four region-2 pair matmuls (start=True, [128,2]
                # strided out) to distinct banks, then the eight region-1
                # accumulate+stop matmuls (RAW pairs spaced >= 4).
                banks = cpb[ep]
                for p0 in range(0, NB2, 4):
                    pg = range(p0, min(p0 + 4, NB2))
                    for p2 in pg:
                        j = p2 // 4
                        cpq = banks[p2 % 4].rearrange(
                            "p (j2 vv) -> p j2 vv", vv=EPOCH)
                        nc.tensor.matmul(
                            cpq[:, 2 * j:2 * j + 2, vloc],
                            g[:, PIX + p2 * 128:PIX + (p2 + 1) * 128], ones_p,
                            start=True, stop=False, skip_group_check=True)
                    for p2 in pg:
                        j = p2 // 4
                        for h, b in ((0, p2), (1, p2 + NB2)):
                            col = (2 * j + h) * EPOCH + vloc
                            nc.tensor.matmul(
                                banks[p2 % 4][:, col:col + 1],
                                g[:, b * 128:(b + 1) * 128], ones_a,
                                start=False, stop=True, skip_group_check=True)

                ep_left[ep] -= 1
                if ep_left[ep] == 0:
                    v0 = ep * EPOCH
                    nv = min(EPOCH, V - v0)
                    for qi in range(4):
                        nj = len(range(qi, NB2, 4))
                        cpv = cpb[ep][qi].rearrange(
                            "p (j h vv) -> p j h vv", h=2, vv=EPOCH)
                        for h in range(2):
                            # cols (2j+h)*EPOCH -> blocks qi + 4j + 18h
                            dst = bass.AP(
                                tensor=cube_sb.tensor,
                                offset=cube_sb.offset
                                + (qi + NB2 * h) * V + v0,
                                ap=[cube_sb.ap[0], [4 * V, nj], [1, nv]])
                            nc.scalar.copy(
                                out=dst, in_=cpv[:, 0:nj, h, 0:nv])
                    del cpb[ep]

            # ---------------- output transpose + store ----------------
            for b in range(NB):
                po = out_psum.tile([V, 128], F32, tag="po")
                nc.tensor.transpose(po, cube_sb[:, b, 0:V], ident)
                so = stage.tile([V, 128], F32, tag="so")
                nc.scalar.copy(out=so, in_=po)
                nc.sync.dma_start(
                    out=cube[:, b * 128:(b + 1) * 128], in_=so)
    return nc


def _make_consts(inclination, sky_rot, velocity_min, velocity_max,
                 line_broadening, v_max, r_t, r_d, h_z, V):
    ci = float(np.cos(np.float32(inclination)))
    si = float(np.sin(np.float32(inclination)))
    cr = float(np.cos(np.float32(sky_rot)))
    sr = float(np.sin(np.float32(sky_rot)))
    sig2 = float(np.float64(line_broadening) ** 2)
    sig = float(np.sqrt(sig2))
    labels = np.linspace(np.float64(velocity_min) / PC,
                         np.float64(velocity_max) / PC, V)
    b_list = (-(labels ** 2) / sig2).astype(np.float64)
    dlab = float(labels[1] - labels[0]) if V > 1 else 0.0
    st2 = 2.0 * dlab / sig2
    # host-side per-channel scale exp(b_v - b_anchor(v)) via the plan
    anchor_of = np.arange(V)
    for (v, kind, src) in _channel_plan(V):
        if kind != "a":
            anchor_of[v] = anchor_of[src]
    cs_list = np.exp(b_list - b_list[anchor_of])
    return {
        "cr": cr, "sr": sr, "ci": ci, "si": si,
        "inv_rd": 1.0 / float(r_d), "inv_hz": 1.0 / float(h_z),
        "rt2": float(r_t) ** 2,
        "cvz": -si * float(v_max),
        "inv_sig": 1.0 / sig,
        "lab_bias": (-labels / sig).astype(np.float64),
        "st2": st2, "cs_list": cs_list,
        "sig2": sig2,
    }


def _run(inputs, trace=False):
    img_x = np.asarray(inputs["img_x"], dtype=np.float32)
    img_y = np.asarray(inputs["img_y"], dtype=np.float32)
    img_z = np.asarray(inputs["img_z"], dtype=np.float32)
    V = int(inputs["velocity_res"])
    consts = _make_consts(
        inputs["inclination"], inputs["sky_rot"], inputs["velocity_min"],
        inputs["velocity_max"], inputs["line_broadening"], inputs["v_max"],
        inputs["r_t"], inputs["r_d"], inputs["h_z"], V)

    nc = bacc.Bacc("TRN2", target_bir_lowering=False, debug=False,
                   num_devices=N_CORES)
    _build(nc, V, consts)
    nc.compile()

    cvec = np.concatenate(
        [[consts["rt2"]], consts["lab_bias"]]).astype(np.float32).reshape(
        1, V + 1)
    in_maps = []
    for c in range(N_CORES):
        sl = slice(c * IPC, (c + 1) * IPC)
        in_maps.append({
            "imx": np.ascontiguousarray(img_x[sl].reshape(PIX, K)),
            "imy": np.ascontiguousarray(img_y[sl].reshape(PIX, K)),
            "imz": np.ascontiguousarray(img_z[sl].reshape(PIX, K)),
            "cvec": cvec,
        })

    res = run_bass_kernel_spmd(
        nc, in_maps, core_ids=list(range(N_CORES)), trace=trace)

    norm = float(inputs["I0"]) / np.sqrt(2.0 * np.pi * consts["sig2"])
    scale = (consts["cs_list"] * norm).astype(np.float64)
    out = np.empty((V, GRID, GRID), dtype=np.float32)
    for c in range(N_CORES):
        shard = res.results[c]["cube"].reshape(V, IPC, GRID).astype(np.float64)
        out[:, c * IPC:(c + 1) * IPC, :] = (
            shard * scale[:, None, None]).astype(np.float32)
    return out, res


def kernel(**inputs):
    out, _ = _run(inputs, trace=False)
    return out


# revision 31
# speedup vs baseline: 1.0021x; 1.0021x over previous
"""Trainium2 Bass kernel for nn_CubeSimulator (galaxy velocity-cube KDE).

Math (matches reference.py exactly, up to fp32 rounding):
    rot_x = img_x*cr - img_y*sr
    y1    = img_x*sr + img_y*cr
    rot_y = y1*ci - img_z*si
    rot_z = y1*si + img_z*ci
    r^2   = rot_x^2 + rot_y^2
    vz    = -si*v_max*rot_x / sqrt(r^2 + r_t^2)     (== -v_y*si of the ref)
    I     = exp(-(sqrt(r^2)/r_d + |rot_z|/h_z))      (I0 applied on host)
    cube[v, pix] = sum_k I * exp(-(lab_v - vz)^2 / sig^2)

Channel structure: anchor channels evaluate G_v = I * exp(-d_v^2/sig^2)
via two ScalarE passes (Square with free affine d = vz/sig - lab_v/sig,
then Exp) and one bf16 VectorE multiply by I; both factors are <= 1 so
nothing overflows and underflow-to-0 matches f32 reference semantics.
Follower channels take 1-2 multiplicative steps G_{v+-1} = G_v * t^{+-1}
with t = exp((2*dlab/sig2)*vz) (bf16 tensor_tensor, DVE 2x mode); the
channel-constant ratio exp(b_v - b_anchor) is applied on the host to the
final [V, pix] cube.  Anchors sit every 5 channels with bidirectional
chains (max 2 steps), so a chain crosses the fp32 underflow boundary by
at most ~e^32, bounding the absolute error at ~1e-24.

Distribution: the leading i-axis (192) is sharded over 8 NeuronCores
(24 planes each -> 4608 sky pixels/core, full k=192 line of sight).
Each core runs the same NEFF; in_maps differ per core.

Per-core layout: k on SBUF partitions, one merged [128, 6912] free axis:
cols 0..4607 = pixels (k 0..127), cols 4608..6911 = pixel pairs
(col c: rows 0:64 = k 128..191 of pixel c-4608, rows 64:128 = pixel
c-4608+2304).  TensorE reduces over k per 128-pixel block (lhsT = G
block as weights, rhs = ones).  PSUM cube tiles rotate over 4 banks so
the region1(start)/region2(accum) same-column pairs are spaced 4 apart
and never stall the PE pipeline; ScalarE drains every 8 channels.
"""

import contextlib

import numpy as np

import concourse.bass as bass
import concourse.bacc as bacc
import concourse.tile as tile
from concourse import mybir
from concourse.alu_op_type import AluOpType
from concourse.bass_utils import run_bass_kernel_spmd
from concourse.masks import make_identity

PC = 3.086e16
GRID = 192
N_CORES = 8
IPC = GRID // N_CORES          # 24 i-planes per core
PIX = IPC * GRID               # 4608 pixels per core
NB = PIX // 128                # 36 pixel blocks
NB2 = NB // 2                  # 18 region-2 pair blocks
K = GRID                       # 192 samples along line of sight
K1 = 128                       # region-1 k rows
PIX2 = PIX // 2                # region-2 pixel-pair columns
MCOL = PIX + PIX2              # merged free width (6912)
EPOCH = 16                     # channels per PSUM drain epoch
ANCHOR = 5                     # anchor spacing (bidirectional chains <= 2)

F32 = mybir.dt.float32
BF16 = mybir.dt.bfloat16
AF = mybir.ActivationFunctionType


def _channel_plan(V):
    """Emission-ordered plan: list of (v, kind, src_v). kind: 'a' anchor,
    'f' forward step (g_v = g_src * t), 'b' backward (g_v = g_src / t).
    Anchors at multiples of ANCHOR (plus a tail anchor); chains <= 2."""
    plan = []
    emitted = set()

    def anchor(v):
        if v not in emitted:
            plan.append((v, "a", None))
            emitted.add(v)

    for m in range(0, V, ANCHOR):
        anchor(m)
        for v in (m + 1, m + 2):
            if v < V and v not in emitted:
                plan.append((v, "f", v - 1))
                emitted.add(v)
        hi = min(m + ANCHOR, V - 1)          # next anchor or tail anchor
        if hi > m + 2:
            anchor(hi)
            for v in range(hi - 1, m + 2, -1):
                if v not in emitted:
                    plan.append((v, "b", v + 1))
                    emitted.add(v)
    assert len(plan) == V and len(emitted) == V
    return plan


def _build(nc, V, consts):
    """Emit the Tile program for one core (same program for all cores)."""
    cr, sr, ci, si = consts["cr"], consts["sr"], consts["ci"], consts["si"]
    inv_rd, inv_hz = consts["inv_rd"], consts["inv_hz"]
    cvz = consts["cvz"]                      # -si*v_max
    inv_sig = consts["inv_sig"]              # 1/sig
    st2 = consts["st2"]                      # 2*dlab/sig2
    assert V <= 64
    plan = _channel_plan(V)

    imx = nc.dram_tensor("imx", [PIX, K], F32, kind="ExternalInput").ap()
    imy = nc.dram_tensor("imy", [PIX, K], F32, kind="ExternalInput").ap()
    imz = nc.dram_tensor("imz", [PIX, K], F32, kind="ExternalInput").ap()
    # cvec: [rt2, -lab_0/sig .. -lab_{V-1}/sig] as per-partition ACT biases
    cvec = nc.dram_tensor("cvec", [1, V + 1], F32, kind="ExternalInput").ap()
    cube = nc.dram_tensor("cube", [V, PIX], F32, kind="ExternalOutput").ap()

    # pixel-major view: [partition(=pixel%128), block, k]
    vx = imx.rearrange("(g p) k -> p g k", p=128)
    vy = imy.rearrange("(g p) k -> p g k", p=128)
    vzr = imz.rearrange("(g p) k -> p g k", p=128)

    with tile.TileContext(nc) as tc:
        ctx = contextlib.ExitStack()
        with ctx:
            singles = ctx.enter_context(tc.tile_pool(name="singles", bufs=1))
            tp_psum = ctx.enter_context(
                tc.tile_pool(name="tp_psum", bufs=1, space="PSUM"))
            c_psum = ctx.enter_context(
                tc.tile_pool(name="c_psum", bufs=1, space="PSUM"))
            out_psum = ctx.enter_context(
                tc.tile_pool(name="out_psum", bufs=2, space="PSUM"))
            stage = ctx.enter_context(tc.tile_pool(name="stage", bufs=3))

            ident = singles.tile([128, 128], F32)
            make_identity(nc, ident)
            ones_a = singles.tile([128, 1], BF16)
            nc.vector.memset(ones_a, 1.0)
            # region-2 pair reduce: col0 sums rows 0:64, col1 rows 64:128
            ones_p = singles.tile([128, 2], BF16)
            nc.vector.memset(ones_p[0:64, 0:1], 1.0)
            nc.vector.memset(ones_p[64:128, 0:1], 0.0)
            nc.vector.memset(ones_p[0:64, 1:2], 0.0)
            nc.vector.memset(ones_p[64:128, 1:2], 1.0)
            cv_sb = singles.tile([128, V + 1], F32)
            nc.sync.dma_start(
                out=cv_sb,
                in_=bass.AP(tensor=cvec.tensor, offset=cvec.offset,
                            ap=[[0, 128]] + cvec.ap[1:]))

            # persistent k-major tensors (merged region1 | region2 layout)
            vzT = singles.tile([128, MCOL], F32)
            iT = singles.tile([128, MCOL], BF16)
            tT = singles.tile([128, MCOL], BF16)
            tiT = singles.tile([128, MCOL], BF16)
            cube_sb = singles.tile([128, NB, V], F32)

            # ---------------- preprocessing (scoped pools) ----------------
            pre_ctx = contextlib.ExitStack()
            with pre_ctx:
                pre2 = pre_ctx.enter_context(tc.tile_pool(name="pre2", bufs=2))
                pre1 = pre_ctx.enter_context(tc.tile_pool(name="pre1", bufs=2))
                CH = 4                       # pixel blocks per chunk
                for c0 in range(0, NB, CH):
                    ng = min(CH, NB - c0)
                    X = pre2.tile([128, ng, K], F32, tag="X")
                    Y = pre2.tile([128, ng, K], F32, tag="Y")
                    Z = pre2.tile([128, ng, K], F32, tag="Z")
                    nc.sync.dma_start(out=X, in_=vx[:, c0:c0 + ng, :])
                    nc.sync.dma_start(out=Y, in_=vy[:, c0:c0 + ng, :])
                    nc.sync.dma_start(out=Z, in_=vzr[:, c0:c0 + ng, :])

                    t0 = pre1.tile([128, ng, K], F32, tag="t0")
                    t1 = pre1.tile([128, ng, K], F32, tag="t1")
                    t2 = pre1.tile([128, ng, K], F32, tag="t2")
                    A = pre1.tile([128, ng, K], F32, tag="A")
                    RY = pre1.tile([128, ng, K], F32, tag="RY")
                    RZ = pre1.tile([128, ng, K], F32, tag="RZ")
                    WI = pre2.tile([128, ng, K], F32, tag="WI")
                    VZ = pre2.tile([128, ng, K], F32, tag="VZ")

                    # A = rot_x = cr*X - sr*Y
                    nc.vector.tensor_scalar_mul(t0, X, float(cr))
                    nc.vector.scalar_tensor_tensor(
                        A, Y, float(-sr), t0, AluOpType.mult, AluOpType.add)
                    # y1 = sr*X + cr*Y   (into t1)
                    nc.vector.tensor_scalar_mul(t2, X, float(sr))
                    nc.vector.scalar_tensor_tensor(
                        t1, Y, float(cr), t2, AluOpType.mult, AluOpType.add)
                    # RY = ci*y1 - si*Z
                    nc.vector.tensor_scalar_mul(t0, t1, float(ci))
                    nc.vector.scalar_tensor_tensor(
                        RY, Z, float(-si), t0, AluOpType.mult, AluOpType.add)
                    # RZ = si*y1 + ci*Z
                    nc.vector.tensor_scalar_mul(t2, t1, float(si))
                    nc.vector.scalar_tensor_tensor(
                        RZ, Z, float(ci), t2, AluOpType.mult, AluOpType.add)
                    # r2 = A^2 + RY^2
                    nc.vector.tensor_mul(t0, A, A)
                    nc.vector.tensor_mul(t1, RY, RY)
                    nc.vector.tensor_add(t0, t0, t1)
                    # Ln group (one ACT table set), then Exp group
                    nc.scalar.activation(
                        t1, t0, AF.Ln, bias=cv_sb[:, 0:1], scale=1.0)
                    nc.scalar.activation(t2, t0, AF.Ln, bias=0.0, scale=1.0)
                    # t1 = rsqrt(r2+rt2); t2 = r = sqrt(r2) (ln(0)->-inf->0)
                    nc.scalar.activation(t1, t1, AF.Exp, bias=0.0, scale=-0.5)
                    nc.scalar.activation(t2, t2, AF.Exp, bias=0.0, scale=0.5)
                    # VZ = cvz*A*rsqrt
                    nc.vector.scalar_tensor_tensor(
                        VZ, A, float(cvz), t1, AluOpType.mult, AluOpType.mult)
                    # t1 = |RZ|/hz ; WI = -(r/rd) - t1   (ln of intensity)
                    nc.scalar.activation(
                        t1, RZ, AF.Abs, bias=0.0, scale=float(inv_hz))
                    nc.vector.scalar_tensor_tensor(
                        WI, t2, float(-inv_rd), t1,
                        AluOpType.mult, AluOpType.subtract)

                    # transpose into the merged k-major tensors; 4 blocks
                    # batch into one PSUM bank and drain with one ScalarE
                    # op, which doubles as Exp for the intensity tensor.
                    for srcT, dT, fn in ((VZ, vzT, AF.Copy),
                                         (WI, iT, AF.Exp)):
                        p1 = tp_psum.tile([128, 4, 128], F32, tag="p1")
                        for gg in range(ng):
                            nc.tensor.transpose(
                                p1[:, gg, :], srcT[:, gg, 0:K1], ident)
                        nc.scalar.activation(
                            dT[:, c0 * 128:(c0 + ng) * 128],
                            p1[:, 0:ng, :], fn, bias=0.0, scale=1.0)
                        p2 = tp_psum.tile([64, 4, 128], F32, tag="p2")
                        for gg in range(ng):
                            nc.tensor.transpose(
                                p2[:, gg, :], srcT[:, gg, K1:K], ident)
                        # region-2 halves: block g -> rows (g<18 ? 0:64),
                        # cols PIX + (g%18)*128; split at the 18 boundary
                        lo = c0
                        while lo < c0 + ng:
                            hi = min(c0 + ng, NB2 if lo < NB2 else NB)
                            r0 = 0 if lo < NB2 else 64
                            gc = PIX + (lo % NB2) * 128
                            nc.scalar.activation(
                                dT[r0:r0 + 64, gc:gc + (hi - lo) * 128],
                                p2[0:64, lo - c0:hi - c0, :], fn,
                                bias=0.0, scale=1.0)
                            lo = hi
                    # channel-step ratio tensors for this chunk's columns
                    for dst, sc in ((tT, st2), (tiT, -st2)):
                        nc.scalar.activation(
                            dst[:, c0 * 128:(c0 + ng) * 128],
                            vzT[:, c0 * 128:(c0 + ng) * 128],
                            AF.Exp, bias=0.0, scale=float(sc))
                        lo = c0
                        while lo < c0 + ng:
                            hi = min(c0 + ng, NB2 if lo < NB2 else NB)
                            r0 = 0 if lo < NB2 else 64
                            gc = PIX + (lo % NB2) * 128
                            nc.scalar.activation(
                                dst[r0:r0 + 64, gc:gc + (hi - lo) * 128],
                                vzT[r0:r0 + 64, gc:gc + (hi - lo) * 128],
                                AF.Exp, bias=0.0, scale=float(sc))
                            lo = hi

            # ---------------- channel loop ----------------
            q_pool = ctx.enter_context(tc.tile_pool(name="q_pool", bufs=1))
            m_pool = ctx.enter_context(tc.tile_pool(name="m_pool", bufs=3))
            g_pool = ctx.enter_context(tc.tile_pool(name="g_pool", bufs=4))

            n_ep = (V + EPOCH - 1) // EPOCH
            ep_left = [min(EPOCH, V - e * EPOCH) for e in range(n_ep)]
            cpb = {}
            g_of = {}
            m_of = {}
            anchors = [v for (v, kind, _s) in plan if kind == "a"]

            def ensure_exp(av):
                if av in m_of:
                    return
                q = q_pool.tile([128, MCOL], F32, tag="q", name=f"q{av}")
                m = m_pool.tile([128, MCOL], BF16, tag="m", name=f"m{av}")
                nc.scalar.activation(
                    q, vzT, AF.Square,
                    bias=cv_sb[:, av + 1:av + 2], scale=float(inv_sig))
                nc.scalar.activation(m, q, AF.Exp, bias=0.0, scale=-1.0)
                m_of[av] = m

            for (v, kind, src) in plan:
                ep = v // EPOCH
                if ep not in cpb:
                    banks = []
                    for qi in range(4):
                        cpq = c_psum.tile([128, 10 * EPOCH], F32,
                                          tag=f"cp{qi}", name=f"cp{qi}")
                        banks.append(cpq)
                    cpb[ep] = banks
                vloc = v % EPOCH
                g = g_pool.tile([128, MCOL], BF16, tag="g")
                if kind == "a":
                    ensure_exp(v)
                    ai = anchors.index(v)
                    if ai + 1 < len(anchors):
                        ensure_exp(anchors[ai + 1])
                    nc.vector.tensor_mul(g, m_of.pop(v), iT)
                else:
                    step = tT if kind == "f" else tiT
                    nc.vector.tensor_mul(g, g_of[src], step)
                g_of[v] = g

                # Pair p2 (blocks p2 and p2+18) -> PSUM bank p2%4, col
                # pair (j=p2//4): half h at col (2j+h)*EPOCH+vloc.  Per
                # 4-group: four region-2 pair matmuls (start=True, [128,2]
                # strided out) to distinct banks, then the eight region-1
                # accumulate+stop matmuls (RAW pairs spaced >= 4).
                banks = cpb[ep]
                for p0 in range(0, NB2, 4):
                    pg = range(p0, min(p0 + 4, NB2))
                    for p2 in pg:
                        j = p2 // 4
                        cpq = banks[p2 % 4].rearrange(
                            "p (j2 vv) -> p j2 vv", vv=EPOCH)
                        nc.tensor.matmul(
                            cpq[:, 2 * j:2 * j + 2, vloc],
                            g[:, PIX + p2 * 128:PIX + (p2 + 1) * 128], ones_p,
                            start=True, stop=False, skip_group_check=True)
                    for p2 in pg:
                        j = p2 // 4
                        for h, b in ((0, p2), (1, p2 + NB2)):
                            col = (2 * j + h) * EPOCH + vloc
                            nc.tensor.matmul(
                                banks[p2 % 4][:, col:col + 1],
                                g[:, b * 128:(b + 1) * 128], ones_a,
                                start=False, stop=True, skip_group_check=True)

                ep_left[ep] -= 1
                if ep_left[ep] == 0:
                    v0 = ep * EPOCH
                    nv = min(EPOCH, V - v0)
                    for qi in range(4):
                        nj = len(range(qi, NB2, 4))
                        cpv = cpb[ep][qi].rearrange(
                            "p (j h vv) -> p j h vv", h=2, vv=EPOCH)
                        for h in range(2):
                            # cols (2j+h)*EPOCH -> blocks qi + 4j + 18h
                            dst = bass.AP(
                                tensor=cube_sb.tensor,
                                offset=cube_sb.offset
                                + (qi + NB2 * h) * V + v0,
                                ap=[cube_sb.ap[0], [4 * V, nj], [1, nv]])
                            nc.scalar.copy(
                                out=dst, in_=cpv[:, 0:nj, h, 0:nv])
                    del cpb[ep]

            # ---------------- output transpose + store ----------------
            for b in range(NB):
                po = out_psum.tile([V, 128], F32, tag="po")
                nc.tensor.transpose(po, cube_sb[:, b, 0:V], ident)
                so = stage.tile([V, 128], F32, tag="so")
                nc.scalar.copy(out=so, in_=po)
                nc.sync.dma_start(
                    out=cube[:, b * 128:(b + 1) * 128], in_=so)
    return nc


def _make_consts(inclination, sky_rot, velocity_min, velocity_max,
                 line_broadening, v_max, r_t, r_d, h_z, V):
    ci = float(np.cos(np.float32(inclination)))
    si = float(np.sin(np.float32(inclination)))
    cr = float(np.cos(np.float32(sky_rot)))
    sr = float(np.sin(np.float32(sky_rot)))
    sig2 = float(np.float64(line_broadening) ** 2)
    sig = float(np.sqrt(sig2))
    labels = np.linspace(np.float64(velocity_min) / PC,
                         np.float64(velocity_max) / PC, V)
    b_list = (-(labels ** 2) / sig2).astype(np.float64)
    dlab = float(labels[1] - labels[0]) if V > 1 else 0.0
    st2 = 2.0 * dlab / sig2
    # host-side per-channel scale exp(b_v - b_anchor(v)) via the plan
    anchor_of = np.arange(V)
    for (v, kind, src) in _channel_plan(V):
        if kind != "a":
            anchor_of[v] = anchor_of[src]
    cs_list = np.exp(b_list - b_list[anchor_of])
    return {
        "cr": cr, "sr": sr, "ci": ci, "si": si,
        "inv_rd": 1.0 / float(r_d), "inv_hz": 1.0 / float(h_z),
        "rt2": float(r_t) ** 2,
        "cvz": -si * float(v_max),
        "inv_sig": 1.0 / sig,
        "lab_bias": (-labels / sig).astype(np.float64),
        "st2": st2, "cs_list": cs_list,
        "sig2": sig2,
    }


def _run(inputs, trace=False):
    img_x = np.asarray(inputs["img_x"], dtype=np.float32)
    img_y = np.asarray(inputs["img_y"], dtype=np.float32)
    img_z = np.asarray(inputs["img_z"], dtype=np.float32)
    V = int(inputs["velocity_res"])
    consts = _make_consts(
        inputs["inclination"], inputs["sky_rot"], inputs["velocity_min"],
        inputs["velocity_max"], inputs["line_broadening"], inputs["v_max"],
        inputs["r_t"], inputs["r_d"], inputs["h_z"], V)

    nc = bacc.Bacc("TRN2", target_bir_lowering=False, debug=False,
                   num_devices=N_CORES)
    _build(nc, V, consts)
    nc.compile()

    cvec = np.concatenate(
        [[consts["rt2"]], consts["lab_bias"]]).astype(np.float32).reshape(
        1, V + 1)
    in_maps = []
    for c in range(N_CORES):
        sl = slice(c * IPC, (c + 1) * IPC)
        in_maps.append({
            "imx": np.ascontiguousarray(img_x[sl].reshape(PIX, K)),
            "imy": np.ascontiguousarray(img_y[sl].reshape(PIX, K)),
            "imz": np.ascontiguousarray(img_z[sl].reshape(PIX, K)),
            "cvec": cvec,
        })

    res = run_bass_kernel_spmd(
        nc, in_maps, core_ids=list(range(N_CORES)), trace=trace)

    norm = float(inputs["I0"]) / np.sqrt(2.0 * np.pi * consts["sig2"])
    scale = (consts["cs_list"] * norm).astype(np.float64)
    out = np.empty((V, GRID, GRID), dtype=np.float32)
    for c in range(N_CORES):
        shard = res.results[c]["cube"].reshape(V, IPC, GRID).astype(np.float64)
        out[:, c * IPC:(c + 1) * IPC, :] = (
            shard * scale[:, None, None]).astype(np.float32)
    return out, res


def kernel(**inputs):
    out, _ = _run(inputs, trace=False)
    return out
